# revision 1
# baseline (speedup 1.0000x reference)
"""CAREConv forward kernel for Trainium2 (8 NeuronCores, Bass/Tile).

Math (per node i with D=32 in-edges grouped by destination):
    t = tanh(feature @ W_mlp.T + b_mlp)            # [N, 2]
    d[i, j] = |t[src[i,j]] - t[i]|.sum()           # L1 dist, [N, D]
    keep K=16 smallest-d in-edges (ties -> lower j, matching lax.top_k)
    h_et[i] = mean_k feature[src[i, keep_k]]       # [N, F]
    out = (0.5 * h_et + feature) @ W_lin.T + b_lin # [N, H]

Distribution: destination nodes sharded over 8 cores (12544 each after
padding 100000 -> 100352).  Each core computes t for its own shard, an
AllGather replicates t to every core, then per-edge t rows and the
selected feature rows are fetched with indirect (gather) DMAs from each
core's full local copy of `feature`.

Selection uses the DVE max8 / match_replace instructions:
  two rounds of (max8 + match_replace) on -d mark the 16 smallest
  distances; masked src ids are then extracted with two more max8
  rounds.  match_replace replaces the FIRST occurrence per value, which
  reproduces lax.top_k's stable tie handling exactly.

On-chip layouts are feature-major ([128 feat partitions, nodes free]) so
both GEMMs run without runtime transposes of the weights; the final
output is produced transposed ([H, shard]) and un-transposed on host.
"""

import numpy as np

import concourse.bacc as bacc
import concourse.bass as bass
import concourse.tile as tile
from concourse import mybir
from concourse.bass import IndirectOffsetOnAxis
from concourse.bass_utils import run_bass_kernel_spmd
from concourse.masks import make_identity
from concourse.tile import add_dep_helper

F32 = mybir.dt.float32
I32 = mybir.dt.int32

# Problem constants (hardcoded per harness contract).
N = 100_000      # real nodes
D = 32           # in-degree
K = 16           # neighbors kept (ceil(D * 0.5))
F = 128          # IN_FEATS
H = 64           # H_FEATS
C = 2            # NUM_CLASSES (t width)
PKEEP = 0.5
NCORES = 8
P = 128          # partitions
SHARD = 12_544   # nodes per core (padded)
NPAD = SHARD * NCORES  # 100352

MINVAL = float(-(2 ** 30))


def build(npad=NPAD, shard=SHARD, ncores=NCORES, tgb=4, bgb=2, debug=False):
    """Build the SPMD Bass program (identical on every core).

    tgb: node-tiles (128 dst nodes each) per t-gather instruction
    bgb: node-tiles per selected-feature gather instruction
    """
    tiles = shard // P
    assert shard % P == 0 and npad == shard * ncores
    assert tgb % bgb == 0

    nc = bacc.Bacc("TRN2", target_bir_lowering=False, debug=False,
                   num_devices=ncores)

    feature = nc.dram_tensor("feature", [npad, F], F32, kind="ExternalInput")
    feat_own = nc.dram_tensor("feat_own", [shard, F], F32, kind="ExternalInput")
    src_own = nc.dram_tensor("src_own", [shard, D], I32, kind="ExternalInput")
    w_mlp_t = nc.dram_tensor("w_mlp_t", [F, C], F32, kind="ExternalInput")
    b_mlp = nc.dram_tensor("b_mlp", [C, 1], F32, kind="ExternalInput")
    w_lin_t = nc.dram_tensor("w_lin_t", [F, H], F32, kind="ExternalInput")
    b_lin = nc.dram_tensor("b_lin", [H, 1], F32, kind="ExternalInput")
    out_t = nc.dram_tensor("out_t", [H, shard], F32, kind="ExternalOutput")

    if debug:
        t_dump = nc.dram_tensor("t_dump", [npad, C], F32, kind="ExternalOutput")
        tsrc_dump = nc.dram_tensor("tsrc_dump", [P, (shard // P) * D * C], F32,
                                   kind="ExternalOutput")
        negd_dump = nc.dram_tensor("negd_dump", [P, (shard // P) * D], F32,
                                   kind="ExternalOutput")
        sel_dump = nc.dram_tensor("sel_dump", [shard, K], I32,
                                  kind="ExternalOutput")
        sidx_dump = nc.dram_tensor("sidx_dump", [P, (shard // P) * D], I32,
                                   kind="ExternalOutput")

    ts = bass.ts

    with tile.TileContext(nc) as tc:
        with (
            tc.tile_pool(name="const", bufs=1) as cpool,
            tc.tile_pool(name="persist", bufs=1) as ppool,
            tc.tile_pool(name="dram", bufs=1, space="DRAM") as dpool,
        ):
            ident = cpool.tile([P, P], F32)
            make_identity(nc, ident[:])
            wm = cpool.tile([F, C], F32)
            nc.sync.dma_start(wm[:], w_mlp_t[:, :])
            wl = cpool.tile([F, H], F32)
            nc.sync.dma_start(wl[:], w_lin_t[:, :])
            bm = cpool.tile([C, 1], F32)
            nc.sync.dma_start(bm[:], b_mlp[:, :])
            bl = cpool.tile([H, 1], F32)
            nc.sync.dma_start(bl[:], b_lin[:, :])

            # Persistent SBUF: transposed own features + negated own t.
            featT = ppool.tile([P, tiles * P], F32)     # [feat, own nodes]
            tneg = ppool.tile([P, tiles * C], F32)      # -t_own, [dst, 2] per tile

            t_shard = dpool.tile([shard, C], F32)
            t_full = dpool.tile([npad, C], F32, addr_space="Shared")
            t_loc = dpool.tile([npad, C], F32)

            # ---------------- Phase 1: t = tanh(feat @ Wmlp.T + b) ----------
            with (
                tc.tile_pool(name="p1", bufs=3) as p1,
                tc.tile_pool(name="p1ps", bufs=2, space="PSUM") as p1ps,
            ):
                for i in range(tiles):
                    ft = p1.tile([P, F], F32, tag="ft")
                    nc.sync.dma_start(ft[:], feat_own[ts(i, P), :])
                    ps_tr = p1ps.tile([P, P], F32, tag="ps_tr")
                    nc.tensor.transpose(ps_tr[:], ft[:], ident[:])
                    nc.scalar.copy(featT[:, ts(i, P)], ps_tr[:])
                    ps_z = p1ps.tile([C, P], F32, tag="ps_z")
                    nc.tensor.matmul(out=ps_z[:], lhsT=wm[:],
                                     rhs=featT[:, ts(i, P)],
                                     start=True, stop=True)
                    tk = p1.tile([C, P], F32, tag="tk")
                    nc.scalar.activation(tk[:], ps_z[:],
                                         mybir.ActivationFunctionType.Tanh,
                                         bias=bm[:, 0:1])
                    # store (component-major AP matches tk's [2, 128] layout)
                    nc.sync.dma_start(
                        t_shard[ts(i, P), :].rearrange("n c -> c n"), tk[:])
                    ps_to = p1ps.tile([P, C], F32, tag="ps_to")
                    nc.tensor.transpose(ps_to[:], tk[:], ident[:C, :C])
                    nc.scalar.mul(tneg[:, ts(i, C)], ps_to[:], -1.0)

            # ---------------- AllGather t across the 8 cores ----------------
            nc.gpsimd.collective_compute(
                "AllGather",
                mybir.AluOpType.bypass,
                replica_groups=[list(range(ncores))],
                ins=[t_shard[:, :]],
                outs=[t_full[:, :]],
            )
            # Copy to a Local tensor; the subsequent indirect gathers read
            # this (regular DMA read of Shared is known-good, and the copy
            # gives the gathers a Tile-trackable producer).
            t_cp = nc.sync.dma_start(t_loc[:, :], t_full[:, :])

            # Hard phase boundary: Tile does not emit DMA-completion waits
            # for the indirect gathers' DRAM-source read, so force a drain
            # (waits for all in-flight DMA data to land) between the t copy
            # and phase 2.
            dr = nc.gpsimd.drain()
            add_dep_helper(dr.ins, t_cp.ins, reason="drain after t copy")

            if debug:
                nc.sync.dma_start(t_dump[:, :], t_loc[:, :])

            # ---------------- Phase 2: select + aggregate + project ---------
            with (
                tc.tile_pool(name="p2", bufs=3) as p2,
                tc.tile_pool(name="p2g", bufs=2) as p2g,
                tc.tile_pool(name="p2ps", bufs=2, space="PSUM") as p2ps,
            ):
                for ib in range(0, tiles, tgb):
                    nb = min(tgb, tiles - ib)
                    # src ids for nb node-tiles: [128, nb*D], tile-major free
                    sidx = p2.tile([P, nb * D], I32, tag="sidx")
                    # [p, (k j)] view of src rows for nb node-tiles:
                    # partition p strides one node row (D), k strides P rows.
                    nc.sync.dma_start(
                        sidx[:],
                        bass.AP(src_own, ib * P * D,
                                [[D, P], [P * D, nb], [1, D]]))
                    # gather t rows per edge: [128, nb*D*2]
                    # Indirect DMA on HW consumes ONE offset per partition
                    # descriptor (dest [128, x] contiguous per partition), so
                    # gather per edge-column: offsets sidx[:, m:m+1].
                    tsrc = p2.tile([P, nb * D * C], F32, tag="tsrc")
                    for m in range(nb * D):
                        tg = nc.gpsimd.indirect_dma_start(
                            out=tsrc[:, m * C:(m + 1) * C], out_offset=None,
                            in_=t_loc[:, :],
                            in_offset=IndirectOffsetOnAxis(
                                ap=sidx[:, m:m + 1], axis=0))
                        add_dep_helper(tg.ins, dr.ins,
                                       reason="t gather after drain")
                    tv = tsrc[:].rearrange("p (m c) -> p m c", c=C)
                    if debug:
                        nc.sync.dma_start(
                            tsrc_dump[:, ib * D * C:(ib + nb) * D * C], tsrc[:])
                        nc.sync.dma_start(
                            sidx_dump[:, ib * D:(ib + nb) * D], sidx[:])

                    for pb in range(0, nb, bgb):
                        npair = min(bgb, nb - pb)
                        selb = p2.tile([P, npair * K], I32, tag="selb")
                        for k2 in range(npair):
                            kk = pb + k2
                            i = ib + kk
                            # -|a_src - a_own|, -|b_src - b_own| via ACT Abs
                            absa = p2.tile([P, D], F32, tag="absa")
                            nc.scalar.activation(
                                absa[:], tv[:, kk * D:(kk + 1) * D, 0],
                                mybir.ActivationFunctionType.Abs,
                                bias=tneg[:, i * C:i * C + 1])
                            absb = p2.tile([P, D], F32, tag="absb")
                            nc.scalar.activation(
                                absb[:], tv[:, kk * D:(kk + 1) * D, 1],
                                mybir.ActivationFunctionType.Abs,
                                bias=tneg[:, i * C + 1:i * C + 2])
                            negd = p2.tile([P, D], F32, tag="negd")
                            nc.vector.scalar_tensor_tensor(
                                out=negd[:], in0=absa[:], scalar=-1.0,
                                in1=absb[:],
                                op0=mybir.AluOpType.mult,
                                op1=mybir.AluOpType.subtract)
                            if debug:
                                nc.sync.dma_start(
                                    negd_dump[:, i * D:(i + 1) * D], negd[:])
                            # two rounds of max8+match_replace -> 16 smallest d
                            v8a = p2.tile([P, 8], F32, tag="v8a")
                            nc.vector.max(v8a[:], negd[:])
                            negd2 = p2.tile([P, D], F32, tag="negd2")
                            nc.vector.match_replace(
                                out=negd2[:], in_to_replace=v8a[:],
                                in_values=negd[:], imm_value=MINVAL)
                            v8b = p2.tile([P, 8], F32, tag="v8b")
                            nc.vector.max(v8b[:], negd2[:])
                            negd3 = p2.tile([P, D], F32, tag="negd3")
                            nc.vector.match_replace(
                                out=negd3[:], in_to_replace=v8b[:],
                                in_values=negd2[:], imm_value=MINVAL)
                            mask = p2.tile([P, D], F32, tag="mask")
                            nc.vector.tensor_scalar(
                                mask[:], negd3[:], MINVAL, None,
                                op0=mybir.AluOpType.is_equal)
                            # masked src+1; extract 16 selected via max8 x2
                            srcf = p2.tile([P, D], F32, tag="srcf")
                            nc.vector.tensor_copy(
                                srcf[:], sidx[:, kk * D:(kk + 1) * D])
                            msrc = p2.tile([P, D], F32, tag="msrc")
                            nc.vector.scalar_tensor_tensor(
                                out=msrc[:], in0=srcf[:], scalar=1.0,
                                in1=mask[:],
                                op0=mybir.AluOpType.add,
                                op1=mybir.AluOpType.mult)
                            self_f = p2.tile([P, K], F32, tag="self_f")
                            nc.vector.max(self_f[:, 0:8], msrc[:])
                            msrc2 = p2.tile([P, D], F32, tag="msrc2")
                            nc.vector.match_replace(
                                out=msrc2[:], in_to_replace=self_f[:, 0:8],
                                in_values=msrc[:], imm_value=0.0)
                            nc.vector.max(self_f[:, 8:16], msrc2[:])
                            nc.vector.tensor_scalar(
                                selb[:, k2 * K:(k2 + 1) * K], self_f[:], 1.0,
                                None, op0=mybir.AluOpType.subtract)

                        if debug:
                            for k2 in range(npair):
                                i = ib + pb + k2
                                nc.sync.dma_start(
                                    sel_dump[ts(i, P), :],
                                    selb[:, k2 * K:(k2 + 1) * K])

                        # gather the selected feature rows: [128, npair*K*F]
                        # (one offset per partition per instruction)
                        fsel = p2g.tile([P, npair * K * F], F32, tag="fsel")
                        for m in range(npair * K):
                            nc.gpsimd.indirect_dma_start(
                                out=fsel[:, m * F:(m + 1) * F], out_offset=None,
                                in_=feature[:, :],
                                in_offset=IndirectOffsetOnAxis(
                                    ap=selb[:, m:m + 1], axis=0))
                        fv = fsel[:].rearrange("p (m f) -> p m f", f=F)

                        for k2 in range(npair):
                            i = ib + pb + k2
                            m0 = k2 * K
                            s8 = p2.tile([P, 8 * F], F32, tag="s8")
                            s8v = s8[:].rearrange("p (m f) -> p m f", f=F)
                            nc.vector.tensor_tensor(
                                out=s8v, in0=fv[:, m0:m0 + 8, :],
                                in1=fv[:, m0 + 8:m0 + 16, :],
                                op=mybir.AluOpType.add)
                            s4 = p2.tile([P, 4 * F], F32, tag="s4")
                            s4v = s4[:].rearrange("p (m f) -> p m f", f=F)
                            nc.vector.tensor_tensor(
                                out=s4v, in0=s8v[:, 0:4, :], in1=s8v[:, 4:8, :],
                                op=mybir.AluOpType.add)
                            s2 = p2.tile([P, 2 * F], F32, tag="s2")
                            s2v = s2[:].rearrange("p (m f) -> p m f", f=F)
                            nc.vector.tensor_tensor(
                                out=s2v, in0=s4v[:, 0:2, :], in1=s4v[:, 2:4, :],
                                op=mybir.AluOpType.add)
                            hsum = p2.tile([P, F], F32, tag="hsum")
                            nc.vector.tensor_tensor(
                                out=hsum[:], in0=s2v[:, 0, :], in1=s2v[:, 1, :],
                                op=mybir.AluOpType.add)
                            # transpose to [feat, dst], fuse residual + scale
                            ps_h = p2ps.tile([P, P], F32, tag="ps_h")
                            nc.tensor.transpose(ps_h[:], hsum[:], ident[:])
                            hT = p2.tile([P, P], F32, tag="hT")
                            nc.vector.scalar_tensor_tensor(
                                out=hT[:], in0=ps_h[:], scalar=PKEEP / K,
                                in1=featT[:, ts(i, P)],
                                op0=mybir.AluOpType.mult,
                                op1=mybir.AluOpType.add)
                            ps_o = p2ps.tile([H, P], F32, tag="ps_o")
                            nc.tensor.matmul(out=ps_o[:], lhsT=wl[:], rhs=hT[:],
                                             start=True, stop=True)
                            ob = p2.tile([H, P], F32, tag="ob")
                            nc.vector.tensor_scalar(
                                ob[:], ps_o[:], bl[:, 0:1], None,
                                op0=mybir.AluOpType.add)
                            nc.sync.dma_start(out_t[:, ts(i, P)], ob[:])

    nc.compile()
    return nc


_NC_CACHE = {}


def _get_nc(debug=False):
    key = (NPAD, SHARD, NCORES, debug)
    if key not in _NC_CACHE:
        _NC_CACHE[key] = build(NPAD, SHARD, NCORES, debug=debug)
    return _NC_CACHE[key]


def make_in_maps(feature, src_ids, W_mlp, b_mlp, W_lin, b_lin,
                 npad=NPAD, shard=SHARD, ncores=NCORES):
    n, f = feature.shape
    fpad = np.zeros((npad, f), np.float32)
    fpad[:n] = np.asarray(feature, np.float32)
    spad = np.zeros((npad * D,), np.int32)
    spad[:src_ids.size] = np.asarray(src_ids, np.int32).ravel()
    src2d = spad.reshape(npad, D)
    wmt = np.ascontiguousarray(np.asarray(W_mlp, np.float32).T)
    wlt = np.ascontiguousarray(np.asarray(W_lin, np.float32).T)
    bm = np.asarray(b_mlp, np.float32).reshape(C, 1)
    bl = np.asarray(b_lin, np.float32).reshape(H, 1)
    in_maps = []
    for c in range(ncores):
        sl = slice(c * shard, (c + 1) * shard)
        in_maps.append({
            "feature": fpad,
            "feat_own": np.ascontiguousarray(fpad[sl]),
            "src_own": np.ascontiguousarray(src2d[sl]),
            "w_mlp_t": wmt,
            "b_mlp": bm,
            "w_lin_t": wlt,
            "b_lin": bl,
        })
    return in_maps


def run(feature, src_ids, W_mlp, b_mlp, W_lin, b_lin, debug=False,
        **spmd_kwargs):
    """Run on hardware; returns (output [N, H] f32, BassKernelResults)."""
    nc = _get_nc(debug=debug)
    in_maps = make_in_maps(feature, src_ids, W_mlp, b_mlp, W_lin, b_lin)
    res = run_bass_kernel_spmd(nc, in_maps, core_ids=list(range(NCORES)),
                               **spmd_kwargs)
    outs = [res.results[c]["out_t"] for c in range(NCORES)]
    full = np.concatenate([o.T for o in outs], axis=0)[:N]
    return np.ascontiguousarray(full, dtype=np.float32), res


def kernel(feature, src_ids, W_mlp, b_mlp, W_lin, b_lin):
    out, _ = run(feature, src_ids, W_mlp, b_mlp, W_lin, b_lin)
    return out



# revision 3
# speedup vs baseline: 1.1308x; 1.1308x over previous
"""CAREConv forward kernel for Trainium2 (8 NeuronCores, Bass/Tile), v2.

Math (per node i with D=32 in-edges grouped by destination):
    t = tanh(feature @ W_mlp.T + b_mlp)            # [N, 2]
    d[i, j] = |t[src[i,j]] - t[i]|.sum()           # L1 dist, [N, D]
    keep K=16 smallest-d in-edges (ties -> lower j, matching lax.top_k)
    h_et[i] = mean_k feature[src[i, keep_k]]       # [N, F]
    out = (0.5 * h_et + feature) @ W_lin.T + b_lin # [N, H]

v2 replaces the baseline's ~4700 per-edge indirect DMAs (each ~1us of
GpSimd SWDGE descriptor generation - which was the entire 5.4ms
baseline runtime) with large-batch dma_gather (InstDMAGatherAnt)
streams plus PE-matmul aggregation:

  - t-gather: one dma_gather per chunk fetches a 256B token per edge
    from a t-table padded to 64B/node (4 nodes/token, token idx =
    src>>2 <= 25087 so int16 indices need no bucketing).  The wanted
    node's (ta, tb) is reduced out with a width-4 one-hot (src&3):
    -d = -reduce_X(onehot * (|ta_s - ta_d| + |tb_s - tb_d|)).
  - selection: the baseline max8/match_replace pipeline (exact
    lax.top_k tie semantics) yields the 16 selected (src+1) values.
  - sel-gather: selected features come from an fp8(e4m3) copy of the
    feature table viewed as 512B quad-rows (4 node rows per token,
    idx = src>>2, single pass).  Each 8-dst slice is aggregated by 4
    PE matmuls whose fp8 lhsT is a quarter-select one-hot mask, so
    the 16-neighbor sums accumulate in PSUM; a transpose + the W_lin
    GEMM + the phase-1 residual projection finish the tile.

Distribution: dst nodes sharded over 8 cores (12544 each, 100000
padded to 100352); each core computes t/fp8 for its shard and two
AllGathers replicate the t-table (6.4MB) and fp8 table (12.8MB).
"""

import numpy as np

import concourse.bacc as bacc
import concourse.bass as bass
import concourse.tile as tile
from concourse import mybir
from concourse.bass_utils import run_bass_kernel_spmd
from concourse.library_config import mlp as mlp_lib
from concourse.masks import make_identity
from concourse.tile import add_dep_helper

F32 = mybir.dt.float32
F8 = mybir.dt.float8e4
I16 = mybir.dt.int16
I32 = mybir.dt.int32

# Problem constants (hardcoded per harness contract).
N = 100_000      # real nodes
D = 32           # in-degree
K = 16           # neighbors kept (ceil(D * 0.5))
F = 128          # IN_FEATS
H = 64           # H_FEATS
C = 2            # NUM_CLASSES (t width)
PKEEP = 0.5
NCORES = 8
P = 128          # partitions
SHARD = 12_544   # nodes per core (padded)
NPAD = SHARD * NCORES  # 100352

TW = 16          # f32 words per node in the padded t table (64B)
TNODE = 4        # nodes per 256B t-gather token
QNODE = 4        # feature rows per 512B fp8 sel-gather token

MINVAL = float(-(2 ** 30))

AF = mybir.ActivationFunctionType
OP = mybir.AluOpType
AX = mybir.AxisListType


def build(npad=NPAD, shard=SHARD, ncores=NCORES, ck=2, stage=99):
    """Build the SPMD Bass program (identical on every core).

    ck: dst node-tiles (128 dsts each) per dma_gather chunk.
    """
    tiles = shard // P
    assert shard % P == 0 and npad == shard * ncores
    assert npad % QNODE == 0 and npad % TNODE == 0
    assert tiles % ck == 0
    nquad = npad // QNODE
    assert nquad <= 2 ** 15, "quad index must fit int16"

    nc = bacc.Bacc("TRN2", target_bir_lowering=False, debug=False,
                   num_devices=ncores)

    feat_own = nc.dram_tensor("feat_own", [shard, F], F32, kind="ExternalInput")
    # t-gather token indices (= src >> 2) int16, stream order
    # i = ((tile*D + j)*128 + p), wrapped [16, shard*D/16] and replicated
    # to all 8 Q7-core partition groups (each core reads its own 16).
    tgidx = nc.dram_tensor("tgidx", [P, shard * D // 16], I16,
                           kind="ExternalInput")
    # src & 3 and src + 1 as f32, [128, tiles*D] (tile-major cols)
    srcw = nc.dram_tensor("srcw", [P, tiles * D], F32, kind="ExternalInput")
    srcp1 = nc.dram_tensor("srcp1", [P, tiles * D], F32, kind="ExternalInput")
    w_mlp_t = nc.dram_tensor("w_mlp_t", [F, C], F32, kind="ExternalInput")
    b_mlp = nc.dram_tensor("b_mlp", [C, 1], F32, kind="ExternalInput")
    w_lin_t = nc.dram_tensor("w_lin_t", [F, H], F32, kind="ExternalInput")
    b_lin = nc.dram_tensor("b_lin", [H, 1], F32, kind="ExternalInput")
    # constants: iota4[p, (j, w)] = w; e16[r, p] = (p%16 == r);
    # bd8[p, j] = (p//16 == j%8)
    iota4 = nc.dram_tensor("iota4", [P, D * TNODE], F32, kind="ExternalInput")
    e16 = nc.dram_tensor("e16", [16, P], F32, kind="ExternalInput")
    bd8 = nc.dram_tensor("bd8", [P, P], F32, kind="ExternalInput")
    out_t = nc.dram_tensor("out_t", [H, shard], F32, kind="ExternalOutput")

    # raw (offset-0) DRAM tensors: dma_gather tables must not live inside
    # a pool arena, and ownp is plain-DMA only.
    t4_loc = nc.dram_tensor("t4_loc", [npad, TW], F32)
    f8_loc = nc.dram_tensor("f8_loc", [npad, F], F8)
    ownp = nc.dram_tensor("ownp", [H, shard], F32)

    ts = bass.ts

    with tile.TileContext(nc) as tc:
        with (
            tc.tile_pool(name="const", bufs=1) as cpool,
            tc.tile_pool(name="persist", bufs=1) as ppool,
            tc.tile_pool(name="dram", bufs=1, space="DRAM") as dpool,
        ):
            # collective operands must be pool-allocated DRAM tiles
            t4_own = dpool.tile([shard, TW], F32)
            f8_own = dpool.tile([shard, F], F8)
            t4_sh = dpool.tile([npad, TW], F32, addr_space="Shared")
            f8_sh = dpool.tile([npad, F], F8, addr_space="Shared")
            ident = cpool.tile([P, P], F32)
            make_identity(nc, ident[:])
            wm = cpool.tile([F, C], F32)
            nc.sync.dma_start(wm[:], w_mlp_t[:, :])
            wl = cpool.tile([F, H], F32)
            nc.sync.dma_start(wl[:], w_lin_t[:, :])
            bm = cpool.tile([C, 1], F32)
            nc.sync.dma_start(bm[:], b_mlp[:, :])
            bl = cpool.tile([H, 1], F32)
            nc.sync.dma_start(bl[:], b_lin[:, :])
            io4 = cpool.tile([P, D * TNODE], F32)
            nc.sync.dma_start(io4[:], iota4[:, :])
            e16t = cpool.tile([16, P], F32)
            nc.sync.dma_start(e16t[:], e16[:, :])
            bdt = cpool.tile([P, P], F32)
            nc.sync.dma_start(bdt[:], bd8[:, :])

            # -t_own components per tile: [128, tiles*2]
            tneg = ppool.tile([P, tiles * C], F32)

            # ---------------- Phase 1: per-shard t, fp8, own projection ----
            with (
                tc.tile_pool(name="p1", bufs=3) as p1,
                tc.tile_pool(name="p1ps", bufs=2, space="PSUM") as p1ps,
            ):
                # zero-fill t4_own (pad words must be 0.0, never NaN)
                zt = p1.tile([P, shard * TW // P], F32, tag="zt")
                nc.vector.memset(zt[:], 0.0)
                z_dma = nc.sync.dma_start(
                    t4_own.rearrange("(p x) w -> p (x w)", p=P), zt[:])
                for i in range(tiles):
                    ft = p1.tile([P, F], F32, tag="ft")
                    nc.sync.dma_start(ft[:], feat_own[ts(i, P), :])
                    # fp8 cast of own features
                    f8t = p1.tile([P, F], F8, tag="f8t")
                    nc.vector.tensor_copy(f8t[:], ft[:])
                    nc.sync.dma_start(f8_own[ts(i, P), :], f8t[:])
                    # transpose -> [feat, node]
                    ps_tr = p1ps.tile([P, P], F32, tag="ps_tr")
                    nc.tensor.transpose(ps_tr[:], ft[:], ident[:])
                    ftT = p1.tile([P, P], F32, tag="ftT")
                    nc.scalar.copy(ftT[:], ps_tr[:])
                    # own projection: wl.T @ ftT + bl -> ownp [H, shard]
                    ps_w = p1ps.tile([H, P], F32, tag="ps_w")
                    nc.tensor.matmul(out=ps_w[:], lhsT=wl[:], rhs=ftT[:],
                                     start=True, stop=True)
                    ow = p1.tile([H, P], F32, tag="ow")
                    nc.scalar.activation(ow[:], ps_w[:], AF.Identity,
                                         bias=bl[:, 0:1])
                    nc.sync.dma_start(ownp[:, ts(i, P)], ow[:])
                    # t = tanh(wm.T @ ftT + bm): [2, 128]
                    ps_z = p1ps.tile([C, P], F32, tag="ps_z")
                    nc.tensor.matmul(out=ps_z[:], lhsT=wm[:], rhs=ftT[:],
                                     start=True, stop=True)
                    tk = p1.tile([C, P], F32, tag="tk")
                    nc.scalar.activation(tk[:], ps_z[:], AF.Tanh,
                                         bias=bm[:, 0:1])
                    # transpose t -> [128, 2]; keep -t for the Abs bias
                    ps_to = p1ps.tile([P, C], F32, tag="ps_to")
                    nc.tensor.transpose(ps_to[:], tk[:], ident[:C, :C])
                    nc.scalar.mul(tneg[:, ts(i, C)], ps_to[:], -1.0)

                # write (ta, tb) into the zeroed t4_own table:
                # node n = (tile i, part p) -> t4_own[128*i+p, 0:2]
                tno = p1.tile([P, tiles * C], F32, tag="tno")
                nc.vector.tensor_scalar(tno[:], tneg[:], -1.0, None,
                                        op0=OP.mult)
                t_dma = nc.gpsimd.dma_start(
                    t4_own.rearrange("(t p) w -> p t w", p=P)[:, :, 0:C],
                    tno[:])
                add_dep_helper(t_dma.ins, z_dma.ins, reason="t4 after zero")

            # ---------------- AllGather t4 + f8 tables ---------------------
            ag_t = nc.gpsimd.collective_compute(
                "AllGather", OP.bypass,
                replica_groups=[list(range(ncores))],
                ins=[t4_own[:, :]], outs=[t4_sh[:, :]],
            )
            add_dep_helper(ag_t.ins, t_dma.ins, reason="AG after t4 write")
            ag_f = nc.gpsimd.collective_compute(
                "AllGather", OP.bypass,
                replica_groups=[list(range(ncores))],
                ins=[f8_own[:, :]], outs=[f8_sh[:, :]],
            )
            t_cp = nc.sync.dma_start(t4_loc[:, :], t4_sh[:, :])
            f_cp = nc.sync.dma_start(f8_loc[:, :], f8_sh[:, :])
            dr = nc.gpsimd.drain()
            add_dep_helper(dr.ins, t_cp.ins, reason="drain after t4 copy")
            add_dep_helper(dr.ins, f_cp.ins, reason="drain after f8 copy")

            if stage <= 1:
                o_dma = nc.sync.dma_start(out_t[:, :], ownp[:, :])
                add_dep_helper(o_dma.ins, dr.ins, reason="stage1 out")
            # gather-table views: [tokens, token-elems]
            t4_tab = t4_loc[:, :].rearrange("(a b) w -> a (b w)", b=TNODE)
            f8_tab = f8_loc[:, :].rearrange("(a b) f -> a (b f)", b=QNODE)
            if stage <= 1:
                tc_skip = True
            else:
                tc_skip = False

            # ---------------- Phase 2: per-chunk gather/select/aggregate ---
            with (
                tc.tile_pool(name="gt", bufs=2) as gt,
                tc.tile_pool(name="gs", bufs=2) as gs,
                tc.tile_pool(name="p2", bufs=3) as p2,
                tc.tile_pool(name="p2ps", bufs=2, space="PSUM") as p2ps,
            ):
                for cb in (range(0, tiles, ck) if not tc_skip else []):
                    nidx_t = ck * D * P
                    nidx_s = ck * K * P
                    # ---- t-gather for ck tiles ----
                    gidx = p2.tile([P, nidx_t // 16], I16, tag="gidx")
                    nc.sync.dma_start(
                        gidx[:],
                        tgidx[:, cb * D * P // 16:
                              (cb + ck) * D * P // 16])
                    gtd = gt.tile([P, ck * D, TW * TNODE], F32, tag="gtd")
                    for k2 in range(ck):
                        g1 = nc.gpsimd.dma_gather(
                            gtd[:, k2 * D:(k2 + 1) * D, :], t4_tab,
                            gidx[:, k2 * D * P // 16:(k2 + 1) * D * P // 16],
                            D * P, D * P, TW * TNODE, single_packet=False)
                        add_dep_helper(g1.ins, dr.ins,
                                       reason="tgather after drain")
                    wsel = p2.tile([P, ck * D], F32, tag="wsel")
                    nc.sync.dma_start(wsel[:], srcw[:, cb * D:(cb + ck) * D])
                    sp1 = p2.tile([P, ck * D], F32, tag="sp1")
                    nc.sync.dma_start(sp1[:], srcp1[:, cb * D:(cb + ck) * D])

                    if stage <= 12:
                        continue
                    # selected-idx arena for this chunk, wrapped [16, .] and
                    # replicated to all 8 Q7-core partition groups
                    qidx = p2.tile([P, ck * P], I16, tag="qidx")
                    # per-tile quarter masks, kept until aggregation
                    lhqs = []

                    for k2 in range(ck):
                        i = cb + k2
                        g4 = gtd[:, k2 * D:(k2 + 1) * D, :].rearrange(
                            "p j (w s) -> p j w s", s=TW)
                        ta = g4[:, :, :, 0:1]
                        tb = g4[:, :, :, 1:2]
                        # one-hot over the 4 nodes of each token
                        oh = p2.tile([P, D, TNODE], F32, tag="oh")
                        wse = wsel[:, k2 * D:(k2 + 1) * D].unsqueeze(2) \
                            .broadcast_to((P, D, TNODE))
                        nc.vector.tensor_tensor(
                            out=oh[:],
                            in0=io4[:].rearrange("p (j w) -> p j w", w=TNODE),
                            in1=wse, op=OP.is_equal)
                        # |ta_src - ta_own| + |tb_src - tb_own| per slot
                        absa = p2.tile([P, D, TNODE], F32, tag="absa")
                        nc.scalar.activation(
                            absa[:].unsqueeze(3), ta, AF.Abs,
                            bias=tneg[:, 2 * i:2 * i + 1])
                        absb = p2.tile([P, D, TNODE], F32, tag="absb")
                        nc.scalar.activation(
                            absb[:].unsqueeze(3), tb, AF.Abs,
                            bias=tneg[:, 2 * i + 1:2 * i + 2])
                        if stage <= 13:
                            continue
                        ab = p2.tile([P, D, TNODE], F32, tag="ab")
                        nc.vector.tensor_tensor(out=ab[:], in0=absa[:],
                                                in1=absb[:], op=OP.add)
                        abm = p2.tile([P, D, TNODE], F32, tag="abm")
                        nc.vector.tensor_tensor(out=abm[:], in0=ab[:],
                                                in1=oh[:], op=OP.mult)
                        negd = p2.tile([P, D], F32, tag="negd")
                        nc.vector.tensor_reduce(negd[:], abm[:], AX.X, OP.add,
                                                negate=True)
                        if stage <= 2:
                            continue

                        # ---- top-16 (lax.top_k tie semantics) ----
                        v8a = p2.tile([P, 8], F32, tag="v8a")
                        nc.vector.max(v8a[:], negd[:])
                        negd2 = p2.tile([P, D], F32, tag="negd2")
                        nc.vector.match_replace(
                            out=negd2[:], in_to_replace=v8a[:],
                            in_values=negd[:], imm_value=MINVAL)
                        v8b = p2.tile([P, 8], F32, tag="v8b")
                        nc.vector.max(v8b[:], negd2[:])
                        negd3 = p2.tile([P, D], F32, tag="negd3")
                        nc.vector.match_replace(
                            out=negd3[:], in_to_replace=v8b[:],
                            in_values=negd2[:], imm_value=MINVAL)
                        mask = p2.tile([P, D], F32, tag="mask")
                        nc.vector.tensor_scalar(
                            mask[:], negd3[:], MINVAL, None, op0=OP.is_equal)
                        # masked (src+1); extract the 16 selected via max8 x2
                        msrc = p2.tile([P, D], F32, tag="msrc")
                        nc.vector.tensor_tensor(
                            out=msrc[:], in0=sp1[:, k2 * D:(k2 + 1) * D],
                            in1=mask[:], op=OP.mult)
                        self_f = p2.tile([P, K], F32, tag="self_f")
                        nc.vector.max(self_f[:, 0:8], msrc[:])
                        msrc2 = p2.tile([P, D], F32, tag="msrc2")
                        nc.vector.match_replace(
                            out=msrc2[:], in_to_replace=self_f[:, 0:8],
                            in_values=msrc[:], imm_value=0.0)
                        nc.vector.max(self_f[:, 8:16], msrc2[:])
                        # sel src = self_f - 1; quad = src>>2; h = src&3
                        sv = p2.tile([P, K], F32, tag="sv")
                        nc.vector.tensor_scalar(sv[:], self_f[:], 1.0, None,
                                                op0=OP.subtract)
                        si = p2.tile([P, K], I32, tag="si")
                        nc.vector.tensor_copy(si[:], sv[:])
                        qi = p2.tile([P, K], I32, tag="qi")
                        nc.vector.tensor_scalar(qi[:], si[:], 2, None,
                                                op0=OP.arith_shift_right)
                        hi = p2.tile([P, K], I32, tag="hi")
                        nc.vector.tensor_scalar(hi[:], si[:], 3, None,
                                                op0=OP.bitwise_and)
                        qf = p2.tile([P, K], F32, tag="qf")
                        nc.vector.tensor_copy(qf[:], qi[:])
                        hf = p2.tile([P, K], F32, tag="hf")
                        nc.vector.tensor_copy(hf[:], hi[:])
                        # transpose to [16, 128]; write idx arena + hT
                        ps_q = p2ps.tile([K, P], F32, tag="ps_q", bufs=1)
                        nc.tensor.transpose(ps_q[:], qf[:], ident[:])
                        qcp = p2.tile([K, P], F32, tag="qcp")
                        nc.scalar.copy(qcp[:], ps_q[:])
                        ps_e2 = p2ps.tile([P, P], F32, tag="ps_e2", bufs=1)
                        nc.tensor.matmul(out=ps_e2[:], lhsT=e16t[:],
                                         rhs=qcp[:], start=True, stop=True)
                        nc.vector.tensor_copy(
                            qidx[:, k2 * P:(k2 + 1) * P], ps_e2[:])
                        ps_h = p2ps.tile([K, P], F32, tag="ps_h", bufs=1)
                        nc.tensor.transpose(ps_h[:], hf[:], ident[:])
                        hT = p2.tile([K, P], F32, tag="hT")
                        nc.scalar.copy(hT[:], ps_h[:])
                        # expand h to slot-partition layout + quarter masks
                        ps_e = p2ps.tile([P, P], F32, tag="ps_e", bufs=1)
                        nc.tensor.matmul(out=ps_e[:], lhsT=e16t[:], rhs=hT[:],
                                         start=True, stop=True)
                        lhq = p2.tile([P, 4 * P], F8, tag=f"lhq{k2}")
                        for c in range(4):
                            nc.vector.scalar_tensor_tensor(
                                out=lhq[:, c * P:(c + 1) * P], in0=ps_e[:],
                                scalar=float(c), in1=bdt[:],
                                op0=OP.is_equal, op1=OP.mult)
                        lhqs.append(lhq)

                    if stage <= 3:
                        continue
                    # ---- selected-feature gather for the chunk ----
                    gsd = gs.tile([P, ck * K, F * QNODE], F8, tag="gsd")
                    g2 = nc.gpsimd.dma_gather(
                        gsd[:], f8_tab, qidx[:], nidx_s, nidx_s, F * QNODE,
                        single_packet=False)
                    add_dep_helper(g2.ins, dr.ins, reason="sgather after drain")

                    if stage <= 4:
                        continue
                    # ---- aggregate + project per tile ----
                    # G-stationary mask-matmuls: lhsT = fp8 feature quarter
                    # [128 slots, 128 feats], rhs = quarter-select mask
                    # [128 slots, 8 dsts] -> ps_t[:, 8s:8s+8] accumulates
                    # h_etT [feat, dst] directly (PSUM col offsets are free).
                    for k2 in range(ck):
                        i = cb + k2
                        lhq = lhqs[k2]
                        ps_t = p2ps.tile([F, P], F32, tag="ps_t")
                        for s in range(16):
                            rhsl = gsd[:, k2 * K + s, :].rearrange(
                                "p (c f) -> p c f", f=F)
                            for c in range(4):
                                nc.tensor.matmul(
                                    out=ps_t[:, 8 * s:8 * s + 8],
                                    lhsT=rhsl[:, c, :],
                                    rhs=lhq[:, c * P + 8 * s:c * P + 8 * s + 8],
                                    start=(c == 0), stop=(c == 3))
                        hsT = p2.tile([F, P], F32, tag="hsT")
                        nc.scalar.copy(hsT[:], ps_t[:])
                        ps_o = p2ps.tile([H, P], F32, tag="ps_o")
                        nc.tensor.matmul(out=ps_o[:], lhsT=wl[:], rhs=hsT[:],
                                         start=True, stop=True)
                        ow2 = p2.tile([H, P], F32, tag="ow2")
                        nc.sync.dma_start(ow2[:], ownp[:, ts(i, P)])
                        ob = p2.tile([H, P], F32, tag="ob")
                        nc.vector.scalar_tensor_tensor(
                            out=ob[:], in0=ps_o[:], scalar=PKEEP / K,
                            in1=ow2[:], op0=OP.mult, op1=OP.add)
                        nc.sync.dma_start(out_t[:, ts(i, P)], ob[:])

    nc.compile()
    return nc


_NC_CACHE = {}


def _get_nc(npad=NPAD, shard=SHARD, ncores=NCORES, ck=2):
    key = (npad, shard, ncores, ck)
    if key not in _NC_CACHE:
        _NC_CACHE[key] = build(npad, shard, ncores, ck=ck)
    return _NC_CACHE[key]


def make_in_maps(feature, src_ids, W_mlp, b_mlp, W_lin, b_lin,
                 npad=NPAD, shard=SHARD, ncores=NCORES):
    n, f = feature.shape
    tiles = shard // P
    fpad = np.zeros((npad, f), np.float32)
    fpad[:n] = np.asarray(feature, np.float32)
    spad = np.zeros((npad * D,), np.int32)
    spad[:src_ids.size] = np.asarray(src_ids, np.int32).ravel()
    src2d = spad.reshape(npad, D)
    wmt = np.ascontiguousarray(np.asarray(W_mlp, np.float32).T)
    wlt = np.ascontiguousarray(np.asarray(W_lin, np.float32).T)
    bm = np.asarray(b_mlp, np.float32).reshape(C, 1)
    bl = np.asarray(b_lin, np.float32).reshape(H, 1)
    iota4 = np.broadcast_to(
        np.tile(np.arange(TNODE, dtype=np.float32), D), (P, D * TNODE))
    iota4 = np.ascontiguousarray(iota4)
    e16 = (np.arange(P)[None, :] % 16 == np.arange(16)[:, None]) \
        .astype(np.float32)
    bd8 = (np.arange(P)[:, None] // 16 == np.arange(P)[None, :] % 8) \
        .astype(np.float32)
    in_maps = []
    for cc in range(ncores):
        sl = slice(cc * shard, (cc + 1) * shard)
        ss = src2d[sl]                              # [shard, D]
        s3 = ss.reshape(tiles, P, D)
        # t-gather idx stream i = ((tile*D + j)*128 + p), wrapped in 16
        # partitions and replicated to all 8 Q7-core groups
        tg = np.ascontiguousarray(np.tile(
            (s3.transpose(0, 2, 1).ravel() >> 2).astype(np.int16)
            .reshape(-1, 16).T, (8, 1)))
        # [128, tiles*D] layouts
        sw = np.ascontiguousarray(
            (s3 & 3).transpose(1, 0, 2).reshape(P, tiles * D)
            .astype(np.float32))
        sp = np.ascontiguousarray(
            (s3 + 1).transpose(1, 0, 2).reshape(P, tiles * D)
            .astype(np.float32))
        in_maps.append({
            "feat_own": np.ascontiguousarray(fpad[sl]),
            "tgidx": tg,
            "srcw": sw,
            "srcp1": sp,
            "w_mlp_t": wmt,
            "b_mlp": bm,
            "w_lin_t": wlt,
            "b_lin": bl,
            "iota4": iota4,
            "e16": e16,
            "bd8": bd8,
        })
    return in_maps


def run(feature, src_ids, W_mlp, b_mlp, W_lin, b_lin, **spmd_kwargs):
    """Run on hardware; returns (output [N, H] f32, BassKernelResults)."""
    nc = _get_nc()
    in_maps = make_in_maps(feature, src_ids, W_mlp, b_mlp, W_lin, b_lin)
    res = run_bass_kernel_spmd(nc, in_maps, core_ids=list(range(NCORES)),
                               **spmd_kwargs)
    outs = [res.results[c]["out_t"] for c in range(NCORES)]
    full = np.concatenate([o.T for o in outs], axis=0)[:N]
    return np.ascontiguousarray(full, dtype=np.float32), res


def kernel(feature, src_ids, W_mlp, b_mlp, W_lin, b_lin):
    out, _ = run(feature, src_ids, W_mlp, b_mlp, W_lin, b_lin)
    return out


# revision 4
# speedup vs baseline: 2.0398x; 1.8039x over previous
"""CAREConv forward kernel for Trainium2 (8 NeuronCores, Bass/Tile), v2.

Math (per node i with D=32 in-edges grouped by destination):
    t = tanh(feature @ W_mlp.T + b_mlp)            # [N, 2]
    d[i, j] = |t[src[i,j]] - t[i]|.sum()           # L1 dist, [N, D]
    keep K=16 smallest-d in-edges (ties -> lower j, matching lax.top_k)
    h_et[i] = mean_k feature[src[i, keep_k]]       # [N, F]
    out = (0.5 * h_et + feature) @ W_lin.T + b_lin # [N, H]

v2 replaces the baseline's ~4700 per-edge indirect DMAs (each ~1us of
GpSimd SWDGE descriptor generation - which was the entire 5.4ms
baseline runtime) with large-batch dma_gather (InstDMAGatherAnt)
streams plus PE-matmul aggregation:

  - t-gather: one dma_gather per chunk fetches a 256B token per edge
    from a t-table padded to 64B/node (4 nodes/token, token idx =
    src>>2 <= 25087 so int16 indices need no bucketing).  The wanted
    node's (ta, tb) is reduced out with a width-4 one-hot (src&3):
    -d = -reduce_X(onehot * (|ta_s - ta_d| + |tb_s - tb_d|)).
  - selection: the baseline max8/match_replace pipeline (exact
    lax.top_k tie semantics) yields the 16 selected (src+1) values.
  - sel-gather: selected features come from an fp8(e4m3) copy of the
    feature table viewed as 512B quad-rows (4 node rows per token,
    idx = src>>2, single pass).  Each 8-dst slice is aggregated by 4
    PE matmuls whose fp8 lhsT is a quarter-select one-hot mask, so
    the 16-neighbor sums accumulate in PSUM; a transpose + the W_lin
    GEMM + the phase-1 residual projection finish the tile.

Distribution: dst nodes sharded over 8 cores (12544 each, 100000
padded to 100352); each core computes t/fp8 for its shard and two
AllGathers replicate the t-table (6.4MB) and fp8 table (12.8MB).
"""

import numpy as np

import concourse.bacc as bacc
import concourse.bass as bass
import concourse.tile as tile
from concourse import mybir
from concourse.bass_utils import run_bass_kernel_spmd
from concourse.library_config import mlp as mlp_lib
from concourse.masks import make_identity
from concourse.tile import add_dep_helper

F32 = mybir.dt.float32
F8 = mybir.dt.float8e4
I16 = mybir.dt.int16
I32 = mybir.dt.int32

# Problem constants (hardcoded per harness contract).
N = 100_000      # real nodes
D = 32           # in-degree
K = 16           # neighbors kept (ceil(D * 0.5))
F = 128          # IN_FEATS
H = 64           # H_FEATS
C = 2            # NUM_CLASSES (t width)
PKEEP = 0.5
NCORES = 8
P = 128          # partitions
SHARD = 12_544   # nodes per core (padded)
NPAD = SHARD * NCORES  # 100352

TW = 16          # f32 words per node in the padded t table (64B)
TNODE = 4        # nodes per 256B t-gather token
QNODE = 4        # feature rows per 512B fp8 sel-gather token

MINVAL = float(-(2 ** 30))

AF = mybir.ActivationFunctionType
OP = mybir.AluOpType
AX = mybir.AxisListType


def build(npad=NPAD, shard=SHARD, ncores=NCORES, ck=2, stage=99):
    """Build the SPMD Bass program (identical on every core).

    ck: dst node-tiles (128 dsts each) per dma_gather chunk.
    """
    tiles = shard // P
    assert shard % P == 0 and npad == shard * ncores
    assert npad % QNODE == 0 and npad % TNODE == 0
    assert tiles % ck == 0
    nquad = npad // QNODE
    assert nquad <= 2 ** 15, "quad index must fit int16"

    nc = bacc.Bacc("TRN2", target_bir_lowering=False, debug=False,
                   num_devices=ncores, num_swdge_queues=4,
                   dynamic_dma_scratch_size=65536)

    feat_own = nc.dram_tensor("feat_own", [shard, F], F32, kind="ExternalInput")
    # t-gather token indices (= src >> 2) int16, stream order
    # i = ((tile*D + j)*128 + p), wrapped [16, shard*D/16] and replicated
    # to all 8 Q7-core partition groups (each core reads its own 16).
    tgidx = nc.dram_tensor("tgidx", [P, shard * D // 16], I16,
                           kind="ExternalInput")
    # src & 3 and src + 1 as f32, [128, tiles*D] (tile-major cols)
    srcw = nc.dram_tensor("srcw", [P, tiles * D], F32, kind="ExternalInput")
    srcp1 = nc.dram_tensor("srcp1", [P, tiles * D], F32, kind="ExternalInput")
    w_mlp_t = nc.dram_tensor("w_mlp_t", [F, C], F32, kind="ExternalInput")
    b_mlp = nc.dram_tensor("b_mlp", [C, 1], F32, kind="ExternalInput")
    w_lin_t = nc.dram_tensor("w_lin_t", [F, H], F32, kind="ExternalInput")
    b_lin = nc.dram_tensor("b_lin", [H, 1], F32, kind="ExternalInput")
    # constants: iota4[p, (j, w)] = w; e16[r, p] = (p%16 == r);
    # bd8[p, j] = (p//16 == j%8)
    iota4 = nc.dram_tensor("iota4", [P, D * TNODE], F32, kind="ExternalInput")
    e16 = nc.dram_tensor("e16", [16, P], F32, kind="ExternalInput")
    bd8 = nc.dram_tensor("bd8", [P, P], F32, kind="ExternalInput")
    out_t = nc.dram_tensor("out_t", [H, shard], F32, kind="ExternalOutput")

    # raw (offset-0) DRAM tensors: dma_gather tables must not live inside
    # a pool arena, and ownp is plain-DMA only.
    t4_loc = nc.dram_tensor("t4_loc", [npad, TW], F32)
    f8_loc = nc.dram_tensor("f8_loc", [npad, F], F8)
    ownp = nc.dram_tensor("ownp", [H, shard], F32)

    ts = bass.ts

    with tile.TileContext(nc) as tc:
        with (
            tc.tile_pool(name="const", bufs=1) as cpool,
            tc.tile_pool(name="persist", bufs=1) as ppool,
            tc.tile_pool(name="dram", bufs=1, space="DRAM") as dpool,
        ):
            # collective operands must be pool-allocated DRAM tiles
            t4_own = dpool.tile([shard, TW], F32)
            f8_own = dpool.tile([shard, F], F8)
            t4_sh = dpool.tile([npad, TW], F32, addr_space="Shared")
            f8_sh = dpool.tile([npad, F], F8, addr_space="Shared")
            ident = cpool.tile([P, P], F32)
            make_identity(nc, ident[:])
            wm = cpool.tile([F, C], F32)
            nc.sync.dma_start(wm[:], w_mlp_t[:, :])
            wl = cpool.tile([F, H], F32)
            nc.sync.dma_start(wl[:], w_lin_t[:, :])
            bm = cpool.tile([C, 1], F32)
            nc.sync.dma_start(bm[:], b_mlp[:, :])
            bl = cpool.tile([H, 1], F32)
            nc.sync.dma_start(bl[:], b_lin[:, :])
            io4 = cpool.tile([P, D * TNODE], F32)
            nc.sync.dma_start(io4[:], iota4[:, :])
            e16t = cpool.tile([16, P], F32)
            nc.sync.dma_start(e16t[:], e16[:, :])
            bdt = cpool.tile([P, P], F32)
            nc.sync.dma_start(bdt[:], bd8[:, :])

            # -t_own components per tile: [128, tiles*2]
            tneg = ppool.tile([P, tiles * C], F32)

            # ---------------- Phase 1: per-shard t, fp8, own projection ----
            with (
                tc.tile_pool(name="p1", bufs=3) as p1,
                tc.tile_pool(name="p1ps", bufs=2, space="PSUM") as p1ps,
            ):
                # zero-fill t4_own (pad words must be 0.0, never NaN)
                zt = p1.tile([P, shard * TW // P], F32, tag="zt")
                nc.vector.memset(zt[:], 0.0)
                z_dma = nc.sync.dma_start(
                    t4_own.rearrange("(p x) w -> p (x w)", p=P), zt[:])
                for i in range(tiles):
                    ft = p1.tile([P, F], F32, tag="ft")
                    nc.sync.dma_start(ft[:], feat_own[ts(i, P), :])
                    # fp8 cast of own features
                    f8t = p1.tile([P, F], F8, tag="f8t")
                    nc.vector.tensor_copy(f8t[:], ft[:])
                    nc.sync.dma_start(f8_own[ts(i, P), :], f8t[:])
                    # transpose -> [feat, node]
                    ps_tr = p1ps.tile([P, P], F32, tag="ps_tr")
                    nc.tensor.transpose(ps_tr[:], ft[:], ident[:])
                    ftT = p1.tile([P, P], F32, tag="ftT")
                    nc.scalar.copy(ftT[:], ps_tr[:])
                    # own projection: wl.T @ ftT + bl -> ownp [H, shard]
                    ps_w = p1ps.tile([H, P], F32, tag="ps_w")
                    nc.tensor.matmul(out=ps_w[:], lhsT=wl[:], rhs=ftT[:],
                                     start=True, stop=True)
                    ow = p1.tile([H, P], F32, tag="ow")
                    nc.scalar.activation(ow[:], ps_w[:], AF.Identity,
                                         bias=bl[:, 0:1])
                    nc.sync.dma_start(ownp[:, ts(i, P)], ow[:])
                    # t = tanh(wm.T @ ftT + bm): [2, 128]
                    ps_z = p1ps.tile([C, P], F32, tag="ps_z")
                    nc.tensor.matmul(out=ps_z[:], lhsT=wm[:], rhs=ftT[:],
                                     start=True, stop=True)
                    tk = p1.tile([C, P], F32, tag="tk")
                    nc.scalar.activation(tk[:], ps_z[:], AF.Tanh,
                                         bias=bm[:, 0:1])
                    # transpose t -> [128, 2]; keep -t for the Abs bias
                    ps_to = p1ps.tile([P, C], F32, tag="ps_to")
                    nc.tensor.transpose(ps_to[:], tk[:], ident[:C, :C])
                    nc.scalar.mul(tneg[:, ts(i, C)], ps_to[:], -1.0)

                # write (ta, tb) into the zeroed t4_own table:
                # node n = (tile i, part p) -> t4_own[128*i+p, 0:2]
                tno = p1.tile([P, tiles * C], F32, tag="tno")
                nc.vector.tensor_scalar(tno[:], tneg[:], -1.0, None,
                                        op0=OP.mult)
                t_dma = nc.gpsimd.dma_start(
                    t4_own.rearrange("(t p) w -> p t w", p=P)[:, :, 0:C],
                    tno[:])
                add_dep_helper(t_dma.ins, z_dma.ins, reason="t4 after zero")

            # ---------------- AllGather t4 + f8 tables ---------------------
            ag_t = nc.gpsimd.collective_compute(
                "AllGather", OP.bypass,
                replica_groups=[list(range(ncores))],
                ins=[t4_own[:, :]], outs=[t4_sh[:, :]],
            )
            add_dep_helper(ag_t.ins, t_dma.ins, reason="AG after t4 write")
            ag_f = nc.gpsimd.collective_compute(
                "AllGather", OP.bypass,
                replica_groups=[list(range(ncores))],
                ins=[f8_own[:, :]], outs=[f8_sh[:, :]],
            )
            t_cp = nc.sync.dma_start(t4_loc[:, :], t4_sh[:, :])
            f_cp = nc.sync.dma_start(f8_loc[:, :], f8_sh[:, :])
            dr = nc.gpsimd.drain()
            add_dep_helper(dr.ins, t_cp.ins, reason="drain after t4 copy")
            add_dep_helper(dr.ins, f_cp.ins, reason="drain after f8 copy")

            if stage <= 1:
                o_dma = nc.sync.dma_start(out_t[:, :], ownp[:, :])
                add_dep_helper(o_dma.ins, dr.ins, reason="stage1 out")
            # gather-table views: [tokens, token-elems]
            t4_tab = t4_loc[:, :].rearrange("(a b) w -> a (b w)", b=TNODE)
            f8_tab = f8_loc[:, :].rearrange("(a b) f -> a (b f)", b=QNODE)
            if stage <= 1:
                tc_skip = True
            else:
                tc_skip = False

            # ---------------- Phase 2: per-chunk gather/select/aggregate ---
            with (
                tc.tile_pool(name="gt", bufs=2) as gt,
                tc.tile_pool(name="gs", bufs=2) as gs,
                tc.tile_pool(name="p2", bufs=3) as p2,
                tc.tile_pool(name="p2ps", bufs=2, space="PSUM") as p2ps,
            ):
                for cb in (range(0, tiles, ck) if not tc_skip else []):
                    nidx_t = ck * D * P
                    nidx_s = ck * K * P
                    # ---- t-gather for ck tiles ----
                    gidx = p2.tile([P, nidx_t // 16], I16, tag="gidx")
                    nc.sync.dma_start(
                        gidx[:],
                        tgidx[:, cb * D * P // 16:
                              (cb + ck) * D * P // 16])
                    gtd = gt.tile([P, ck * D, TW * TNODE], F32, tag="gtd")
                    for k2 in range(ck):
                        g1 = nc.gpsimd.dma_gather(
                            gtd[:, k2 * D:(k2 + 1) * D, :], t4_tab,
                            gidx[:, k2 * D * P // 16:(k2 + 1) * D * P // 16],
                            D * P, D * P, TW * TNODE, single_packet=False,
                            queue_num=(cb + k2) % 4)
                        add_dep_helper(g1.ins, dr.ins,
                                       reason="tgather after drain")
                    wsel = p2.tile([P, ck * D], F32, tag="wsel")
                    nc.sync.dma_start(wsel[:], srcw[:, cb * D:(cb + ck) * D])
                    sp1 = p2.tile([P, ck * D], F32, tag="sp1")
                    nc.sync.dma_start(sp1[:], srcp1[:, cb * D:(cb + ck) * D])

                    if stage <= 12:
                        continue
                    # selected-idx arena for this chunk, wrapped [16, .] and
                    # replicated to all 8 Q7-core partition groups
                    qidx = p2.tile([P, ck * P], I16, tag="qidx")
                    # per-tile quarter masks, kept until aggregation
                    lhqs = []

                    for k2 in range(ck):
                        i = cb + k2
                        g4 = gtd[:, k2 * D:(k2 + 1) * D, :].rearrange(
                            "p j (w s) -> p j w s", s=TW)
                        ta = g4[:, :, :, 0:1]
                        tb = g4[:, :, :, 1:2]
                        # one-hot over the 4 nodes of each token
                        oh = p2.tile([P, D, TNODE], F32, tag="oh")
                        wse = wsel[:, k2 * D:(k2 + 1) * D].unsqueeze(2) \
                            .broadcast_to((P, D, TNODE))
                        nc.vector.tensor_tensor(
                            out=oh[:],
                            in0=io4[:].rearrange("p (j w) -> p j w", w=TNODE),
                            in1=wse, op=OP.is_equal)
                        # |ta_src - ta_own| + |tb_src - tb_own| per slot
                        absa = p2.tile([P, D, TNODE], F32, tag="absa")
                        nc.scalar.activation(
                            absa[:].unsqueeze(3), ta, AF.Abs,
                            bias=tneg[:, 2 * i:2 * i + 1])
                        absb = p2.tile([P, D, TNODE], F32, tag="absb")
                        nc.scalar.activation(
                            absb[:].unsqueeze(3), tb, AF.Abs,
                            bias=tneg[:, 2 * i + 1:2 * i + 2])
                        if stage <= 13:
                            continue
                        ab = p2.tile([P, D, TNODE], F32, tag="ab")
                        nc.vector.tensor_tensor(out=ab[:], in0=absa[:],
                                                in1=absb[:], op=OP.add)
                        abm = p2.tile([P, D, TNODE], F32, tag="abm")
                        nc.vector.tensor_tensor(out=abm[:], in0=ab[:],
                                                in1=oh[:], op=OP.mult)
                        negd = p2.tile([P, D], F32, tag="negd")
                        nc.vector.tensor_reduce(negd[:], abm[:], AX.X, OP.add,
                                                negate=True)
                        if stage <= 2:
                            continue

                        # ---- top-16 (lax.top_k tie semantics) ----
                        v8a = p2.tile([P, 8], F32, tag="v8a")
                        nc.vector.max(v8a[:], negd[:])
                        negd2 = p2.tile([P, D], F32, tag="negd2")
                        nc.vector.match_replace(
                            out=negd2[:], in_to_replace=v8a[:],
                            in_values=negd[:], imm_value=MINVAL)
                        v8b = p2.tile([P, 8], F32, tag="v8b")
                        nc.vector.max(v8b[:], negd2[:])
                        negd3 = p2.tile([P, D], F32, tag="negd3")
                        nc.vector.match_replace(
                            out=negd3[:], in_to_replace=v8b[:],
                            in_values=negd2[:], imm_value=MINVAL)
                        mask = p2.tile([P, D], F32, tag="mask")
                        nc.vector.tensor_scalar(
                            mask[:], negd3[:], MINVAL, None, op0=OP.is_equal)
                        # masked (src+1); extract the 16 selected via max8 x2
                        msrc = p2.tile([P, D], F32, tag="msrc")
                        nc.vector.tensor_tensor(
                            out=msrc[:], in0=sp1[:, k2 * D:(k2 + 1) * D],
                            in1=mask[:], op=OP.mult)
                        self_f = p2.tile([P, K], F32, tag="self_f")
                        nc.vector.max(self_f[:, 0:8], msrc[:])
                        msrc2 = p2.tile([P, D], F32, tag="msrc2")
                        nc.vector.match_replace(
                            out=msrc2[:], in_to_replace=self_f[:, 0:8],
                            in_values=msrc[:], imm_value=0.0)
                        nc.vector.max(self_f[:, 8:16], msrc2[:])
                        # sel src = self_f - 1; quad = src>>2; h = src&3
                        sv = p2.tile([P, K], F32, tag="sv")
                        nc.vector.tensor_scalar(sv[:], self_f[:], 1.0, None,
                                                op0=OP.subtract)
                        si = p2.tile([P, K], I32, tag="si")
                        nc.vector.tensor_copy(si[:], sv[:])
                        qi = p2.tile([P, K], I32, tag="qi")
                        nc.vector.tensor_scalar(qi[:], si[:], 2, None,
                                                op0=OP.arith_shift_right)
                        hi = p2.tile([P, K], I32, tag="hi")
                        nc.vector.tensor_scalar(hi[:], si[:], 3, None,
                                                op0=OP.bitwise_and)
                        qf = p2.tile([P, K], F32, tag="qf")
                        nc.vector.tensor_copy(qf[:], qi[:])
                        hf = p2.tile([P, K], F32, tag="hf")
                        nc.vector.tensor_copy(hf[:], hi[:])
                        # transpose to [16, 128]; write idx arena + hT
                        ps_q = p2ps.tile([K, P], F32, tag="ps_q", bufs=1)
                        nc.tensor.transpose(ps_q[:], qf[:], ident[:])
                        qcp = p2.tile([K, P], F32, tag="qcp")
                        nc.scalar.copy(qcp[:], ps_q[:])
                        ps_e2 = p2ps.tile([P, P], F32, tag="ps_e2", bufs=1)
                        nc.tensor.matmul(out=ps_e2[:], lhsT=e16t[:],
                                         rhs=qcp[:], start=True, stop=True)
                        nc.vector.tensor_copy(
                            qidx[:, k2 * P:(k2 + 1) * P], ps_e2[:])
                        ps_h = p2ps.tile([K, P], F32, tag="ps_h", bufs=1)
                        nc.tensor.transpose(ps_h[:], hf[:], ident[:])
                        hT = p2.tile([K, P], F32, tag="hT")
                        nc.scalar.copy(hT[:], ps_h[:])
                        # expand h to slot-partition layout + quarter masks
                        ps_e = p2ps.tile([P, P], F32, tag="ps_e", bufs=1)
                        nc.tensor.matmul(out=ps_e[:], lhsT=e16t[:], rhs=hT[:],
                                         start=True, stop=True)
                        lhq = p2.tile([P, 4 * P], F8, tag=f"lhq{k2}")
                        for c in range(4):
                            nc.vector.scalar_tensor_tensor(
                                out=lhq[:, c * P:(c + 1) * P], in0=ps_e[:],
                                scalar=float(c), in1=bdt[:],
                                op0=OP.is_equal, op1=OP.mult)
                        lhqs.append(lhq)

                    if stage <= 3:
                        continue
                    # ---- selected-feature gather for the chunk ----
                    gsd = gs.tile([P, ck * K, F * QNODE], F8, tag="gsd")
                    g2 = nc.gpsimd.dma_gather(
                        gsd[:], f8_tab, qidx[:], nidx_s, nidx_s, F * QNODE,
                        single_packet=False, queue_num=(cb // ck) % 4)
                    add_dep_helper(g2.ins, dr.ins, reason="sgather after drain")

                    if stage <= 4:
                        continue
                    # ---- aggregate + project per tile ----
                    # G-stationary mask-matmuls: lhsT = fp8 feature quarter
                    # [128 slots, 128 feats], rhs = quarter-select mask
                    # [128 slots, 8 dsts] -> ps_t[:, 8s:8s+8] accumulates
                    # h_etT [feat, dst] directly (PSUM col offsets are free).
                    for k2 in range(ck):
                        i = cb + k2
                        lhq = lhqs[k2]
                        ps_t = p2ps.tile([F, P], F32, tag="ps_t")
                        for s in range(16):
                            rhsl = gsd[:, k2 * K + s, :].rearrange(
                                "p (c f) -> p c f", f=F)
                            for c in range(4):
                                nc.tensor.matmul(
                                    out=ps_t[:, 8 * s:8 * s + 8],
                                    lhsT=rhsl[:, c, :],
                                    rhs=lhq[:, c * P + 8 * s:c * P + 8 * s + 8],
                                    start=(c == 0), stop=(c == 3))
                        hsT = p2.tile([F, P], F32, tag="hsT")
                        nc.scalar.copy(hsT[:], ps_t[:])
                        ps_o = p2ps.tile([H, P], F32, tag="ps_o")
                        nc.tensor.matmul(out=ps_o[:], lhsT=wl[:], rhs=hsT[:],
                                         start=True, stop=True)
                        ow2 = p2.tile([H, P], F32, tag="ow2")
                        nc.sync.dma_start(ow2[:], ownp[:, ts(i, P)])
                        ob = p2.tile([H, P], F32, tag="ob")
                        nc.vector.scalar_tensor_tensor(
                            out=ob[:], in0=ps_o[:], scalar=PKEEP / K,
                            in1=ow2[:], op0=OP.mult, op1=OP.add)
                        nc.sync.dma_start(out_t[:, ts(i, P)], ob[:])

    nc.compile()
    return nc


_NC_CACHE = {}


def _get_nc(npad=NPAD, shard=SHARD, ncores=NCORES, ck=2):
    key = (npad, shard, ncores, ck)
    if key not in _NC_CACHE:
        _NC_CACHE[key] = build(npad, shard, ncores, ck=ck)
    return _NC_CACHE[key]


def make_in_maps(feature, src_ids, W_mlp, b_mlp, W_lin, b_lin,
                 npad=NPAD, shard=SHARD, ncores=NCORES):
    n, f = feature.shape
    tiles = shard // P
    fpad = np.zeros((npad, f), np.float32)
    fpad[:n] = np.asarray(feature, np.float32)
    spad = np.zeros((npad * D,), np.int32)
    spad[:src_ids.size] = np.asarray(src_ids, np.int32).ravel()
    src2d = spad.reshape(npad, D)
    wmt = np.ascontiguousarray(np.asarray(W_mlp, np.float32).T)
    wlt = np.ascontiguousarray(np.asarray(W_lin, np.float32).T)
    bm = np.asarray(b_mlp, np.float32).reshape(C, 1)
    bl = np.asarray(b_lin, np.float32).reshape(H, 1)
    iota4 = np.broadcast_to(
        np.tile(np.arange(TNODE, dtype=np.float32), D), (P, D * TNODE))
    iota4 = np.ascontiguousarray(iota4)
    e16 = (np.arange(P)[None, :] % 16 == np.arange(16)[:, None]) \
        .astype(np.float32)
    bd8 = (np.arange(P)[:, None] // 16 == np.arange(P)[None, :] % 8) \
        .astype(np.float32)
    in_maps = []
    for cc in range(ncores):
        sl = slice(cc * shard, (cc + 1) * shard)
        ss = src2d[sl]                              # [shard, D]
        s3 = ss.reshape(tiles, P, D)
        # t-gather idx stream i = ((tile*D + j)*128 + p), wrapped in 16
        # partitions and replicated to all 8 Q7-core groups
        tg = np.ascontiguousarray(np.tile(
            (s3.transpose(0, 2, 1).ravel() >> 2).astype(np.int16)
            .reshape(-1, 16).T, (8, 1)))
        # [128, tiles*D] layouts
        sw = np.ascontiguousarray(
            (s3 & 3).transpose(1, 0, 2).reshape(P, tiles * D)
            .astype(np.float32))
        sp = np.ascontiguousarray(
            (s3 + 1).transpose(1, 0, 2).reshape(P, tiles * D)
            .astype(np.float32))
        in_maps.append({
            "feat_own": np.ascontiguousarray(fpad[sl]),
            "tgidx": tg,
            "srcw": sw,
            "srcp1": sp,
            "w_mlp_t": wmt,
            "b_mlp": bm,
            "w_lin_t": wlt,
            "b_lin": bl,
            "iota4": iota4,
            "e16": e16,
            "bd8": bd8,
        })
    return in_maps


def run(feature, src_ids, W_mlp, b_mlp, W_lin, b_lin, **spmd_kwargs):
    """Run on hardware; returns (output [N, H] f32, BassKernelResults)."""
    nc = _get_nc()
    in_maps = make_in_maps(feature, src_ids, W_mlp, b_mlp, W_lin, b_lin)
    res = run_bass_kernel_spmd(nc, in_maps, core_ids=list(range(NCORES)),
                               **spmd_kwargs)
    outs = [res.results[c]["out_t"] for c in range(NCORES)]
    full = np.concatenate([o.T for o in outs], axis=0)[:N]
    return np.ascontiguousarray(full, dtype=np.float32), res


def kernel(feature, src_ids, W_mlp, b_mlp, W_lin, b_lin):
    out, _ = run(feature, src_ids, W_mlp, b_mlp, W_lin, b_lin)
    return out
